# revision 5
# baseline (speedup 1.0000x reference)
"""Trainium2 kernel for nn_InterpolatorMaskArgs (embedding_lookup, memory regime).

reference computes:  ind = floor((x[0]-X0)/DX);  res = sum(roll(mask, ind) * yOrig)
i.e. a full O(N) dot product between yOrig and the rolled mask, with an
out-of-range guard on x.

Strategy (matches the sharding hint):
  - 1-D shard yOrig along N across the 8 cores (contiguous 2M-element shards).
  - The roll is resolved at shard time: core c receives the slice of the
    rolled mask aligned with its yOrig shard, i.e. mask[(c*S - ind) mod N ...]
    (mod-N wraparound == the halo exchange, done while scattering inputs).
  - Inputs are cast to fp16 on the host (tolerance is 2e-2; measured end-to-end
    rel err ~1e-3), halving HBM traffic vs fp32: 8 MiB/core @ ~358 GB/s.
  - Host packs y and mask tile-interleaved into one fp16 [P, NT*2T] tensor so
    each SBUF tile arrives via a single DMA of 128 contiguous 8 KiB rows.
  - Tile DMAs alternate between two descriptor-generation queues (sync HWDGE
    and gpsimd SWDGE): consecutive transfers on ONE ring serialize their
    ~1.6 us completion-receipt overhead, two rings overlap it and keep the
    16 SDMA engines at HBM line rate.
  - Per tile on VectorE: fp16 in-place product (DVE 2x mode) + free-dim
    tensor_reduce into fp32 acc[:, i].  ScalarE stays off the critical path
    (this build rejects the fused tensor_tensor_reduce ISA encoding).
  - The final all-reduce of the 8*128*NT fp32 partials is done on the host
    (a few KB), followed by the out-of-range predicate.
"""

import numpy as np

import concourse.bass as bass
import concourse.mybir as mybir
from concourse.bass_utils import run_bass_kernel_spmd

# Grid constants (must match the problem's reference.py)
N = 16777216
X0 = 0.0
DX = 1.0
XMAX = X0 + (N - 1) * DX

NCORES = 8
P = 128                 # SBUF partitions
S = N // NCORES         # 2,097,152 elements per core
F = S // P              # 16,384 free-dim elements per partition
T = 2048                # tile free width per half
NT = F // T             # tiles per shard
W = 2 * T               # packed row width per tile (y half + m half)

_CACHED_NC = None
NB = 6                  # SBUF buffer slots


def _build_nc():
    """Raw Bass (not Tile): this walrus build rejects instructions carrying
    more than ~1 inline semaphore wait ("Too many sync wait commands"), so
    all cross-engine sync uses standalone wait_ge instructions."""
    nc = bass.Bass(trn_type="TRN2")
    f16 = mybir.dt.float16
    f32 = mybir.dt.float32
    ym = nc.dram_tensor("ym", [P, NT * W], f16, kind="ExternalInput")
    out = nc.dram_tensor("out", [P, NT], f32, kind="ExternalOutput")

    with (
        nc.Block() as block,
        nc.semaphore("dma0") as d0,
        nc.semaphore("dma1") as d1,
        nc.semaphore("dma2") as d2,
        nc.semaphore("dma3") as d3,
        nc.semaphore("dma4") as d4,
        nc.semaphore("dma5") as d5,
        nc.semaphore("vec_sem") as vec_sem,
        nc.semaphore("out_sem") as out_sem,
        nc.sbuf_tensor("ct0", [P, W], f16) as ct0,
        nc.sbuf_tensor("ct1", [P, W], f16) as ct1,
        nc.sbuf_tensor("ct2", [P, W], f16) as ct2,
        nc.sbuf_tensor("ct3", [P, W], f16) as ct3,
        nc.sbuf_tensor("ct4", [P, W], f16) as ct4,
        nc.sbuf_tensor("ct5", [P, W], f16) as ct5,
        nc.sbuf_tensor("acc", [P, NT], f32) as acc,
    ):
        dsems = [d0, d1, d2, d3, d4, d5]
        cts = [ct0, ct1, ct2, ct3, ct4, ct5]

        def issue(eng, i):
            b = i % NB
            if i >= NB:
                # slot reuse: wait until the reduce of tile i-NB released it
                eng.wait_ge(vec_sem, i - NB + 1)
            eng.dma_start(
                out=cts[b][:], in_=ym[:, i * W:(i + 1) * W]
            ).then_inc(dsems[b], 16)

        @block.sync
        def _(sync):
            for i in range(0, NT, 2):
                issue(sync, i)
            sync.wait_ge(vec_sem, NT)
            sync.dma_start(out=out[:], in_=acc[:]).then_inc(out_sem, 16)
            sync.wait_ge(out_sem, 16)

        @block.gpsimd
        def _(gpsimd):
            for i in range(1, NT, 2):
                issue(gpsimd, i)

        @block.vector
        def _(vector):
            for i in range(NT):
                b = i % NB
                vector.wait_ge(dsems[b], 16 * (i // NB + 1))
                # in-place product into the y half (fp16 -> DVE 2x mode)
                nc.vector.tensor_mul(
                    out=cts[b][:, 0:T], in0=cts[b][:, 0:T], in1=cts[b][:, T:W]
                )
                # acc[:, i] = per-partition free-dim sum of the product
                nc.vector.tensor_reduce(
                    out=acc[:, i:i + 1],
                    in_=cts[b][:, 0:T],
                    axis=mybir.AxisListType.X,
                    op=mybir.AluOpType.add,
                ).then_inc(vec_sem, 1)

    return nc


def _get_nc():
    global _CACHED_NC
    if _CACHED_NC is None:
        _CACHED_NC = _build_nc()
    return _CACHED_NC


def kernel(x, yOrig, mask):
    x = np.asarray(x)
    yOrig = np.asarray(yOrig, dtype=np.float32)
    mask = np.asarray(mask, dtype=np.float32)

    xs = float(x.reshape(-1)[0])
    ind = int(np.floor((xs - X0) / DX))
    shift = ind % N

    # rolled[i] = mask[(i - ind) mod N]  (== np.roll(mask, ind))
    if shift == 0:
        rolled = mask
    else:
        rolled = np.concatenate([mask[N - shift:], mask[:N - shift]])

    yq = yOrig.astype(np.float16)
    mq = rolled.astype(np.float16)

    in_maps = []
    for c in range(NCORES):
        ymc = np.empty((P, NT, 2, T), dtype=np.float16)
        ymc[:, :, 0, :] = yq[c * S:(c + 1) * S].reshape(P, NT, T)
        ymc[:, :, 1, :] = mq[c * S:(c + 1) * S].reshape(P, NT, T)
        in_maps.append({"ym": ymc.reshape(P, NT * W)})

    res = run_bass_kernel_spmd(_get_nc(), in_maps, core_ids=list(range(NCORES)))

    partials = np.concatenate([r["out"].reshape(-1) for r in res.results])
    total = np.float32(partials.sum(dtype=np.float32))

    if xs >= XMAX or xs < X0:
        total = np.float32(0.0)

    # Stash for test harnesses that want profiling info.
    kernel.last_results = res
    return np.asarray(total, dtype=np.float32)
